# revision 5
# baseline (speedup 1.0000x reference)
"""DNA-Net GNN message passing on 8 Trainium2 NeuronCores.

Strategy (per spec sharding hint): edges are partitioned across the 8 cores.
Edges are sorted by target (col); each 128-node target block's edges are split
evenly across cores so the SPMD program structure is identical on every core.
Per layer each core gathers Q[col]/K[row]/V[row] rows from replicated DRAM
tables via dma_gather, computes per-edge attention on-chip, aggregates per
target block with one-hot matmuls into PSUM, and the per-core partial node
sums are combined with ReduceScatter(+)/AllGather collectives.

All floating-point math runs on device. Host-side work is limited to index
preprocessing (self-loops, sort, shard, pad), weight layout permutation, and
sharding/unsharding of inputs/outputs.
"""

import math
import numpy as np

try:
    from ml_dtypes import bfloat16 as np_bf16
except ImportError:  # pragma: no cover
    np_bf16 = None

# ---------------------------------------------------------------- constants
N = 25000
E0 = 400000
HEADS = 8
DH = 8
HID = 64
F_IN = 256
N_CLASS = 32
N_LAYERS = 3
NCORES = 8

BLK = 128          # target node block size
NPAD = 25600       # N padded to BLK*NB
NB = NPAD // BLK   # 200 blocks
SHARD = NPAD // NCORES      # 3200 nodes per core
SBLK = SHARD // BLK         # 25 blocks per core shard
BATCH = 4096       # edges per gather batch

# stored feature order: position d*8+h holds original feature h*8+d
PERM = np.arange(64).reshape(8, 8).T.ravel()   # PERM[d*8+h] = h*8+d
ISQ = 1.0 / math.sqrt(DH)


# ---------------------------------------------------------------- host prep
def _block_diag(w):
    """w: [G, 8, 8] -> [64, 64] block diagonal."""
    G = w.shape[0]
    out = np.zeros((64, 64), np.float32)
    for g in range(G):
        out[8 * g:8 * g + 8, 8 * g:8 * g + 8] = w[g]
    return out


def _perm_w(bd):
    """apply stored-order permutation to rows+cols of a [64,64] matrix."""
    return bd[PERM][:, PERM]


def _wrap16(idx, dtype=np.int16):
    """dma_gather index layout: [128, E/16]; idx i at [i%16 + 16r, i//16]."""
    E = idx.shape[0]
    assert E % 16 == 0
    w = idx.astype(dtype).reshape(E // 16, 16).T  # [16, E/16]
    return np.tile(w, (8, 1))  # replicate across the 8 Q7 cores -> [128, E/16]


def prep_edges(edge_index):
    """Build per-core edge streams + shared matmul-set structure.

    Returns dict with:
      rows   [NCORES, EPAD] int32   source node (pad: 0)
      cols   [NCORES, EPAD] int32   target node (pad: -1)
      sets   list of (group g, block b, first_of_block, last_of_block)
      EPAD   stream length (multiple of 128, same on all cores)
    """
    row = np.concatenate([edge_index[0], np.arange(N)]).astype(np.int64)
    col = np.concatenate([edge_index[1], np.arange(N)]).astype(np.int64)
    order = np.argsort(col, kind="stable")
    row, col = row[order], col[order]

    Tb = np.bincount(col // BLK, minlength=NB)          # edges per block
    Ub = np.maximum((Tb + NCORES - 1) // NCORES, 1)     # per-core slots/block
    Sb = np.concatenate([[0], np.cumsum(Ub)])           # stream offsets
    total = int(Sb[-1])
    EPAD = ((total + BLK - 1) // BLK) * BLK

    rows = np.zeros((NCORES, EPAD), np.int32)
    cols = np.full((NCORES, EPAD), -1, np.int32)
    bstart = np.concatenate([[0], np.cumsum(Tb)])
    for b in range(NB):
        t = int(Tb[b])
        base, rem = divmod(t, NCORES)
        for c in range(NCORES):
            cnt = base + (1 if c < rem else 0)
            off = c * base + min(c, rem)
            s = int(Sb[b])
            rows[c, s:s + cnt] = row[bstart[b] + off: bstart[b] + off + cnt]
            cols[c, s:s + cnt] = col[bstart[b] + off: bstart[b] + off + cnt]

    # matmul sets: for each 128-edge group, each overlapped block
    sets = []
    NG = EPAD // BLK
    for b in range(NB):
        g0 = int(Sb[b]) // BLK
        g1 = (int(Sb[b + 1]) - 1) // BLK
        for g in range(g0, g1 + 1):
            sets.append((g, b, g == g0, g == g1))
    return dict(rows=rows, cols=cols, sets=sets, EPAD=EPAD, NG=NG, Ub=Ub, Sb=Sb)


def prep_weights(lin1_w, lin1_b, wq, bq, wk, bk, wv, bv, lin2_w, lin2_b):
    """Host-side layout permutation of the (tiny) weights."""
    out = {}
    out["W1"] = lin1_w[:, PERM].astype(np.float32)          # [256, 64]
    out["b1"] = lin1_b[PERM].astype(np.float32)
    for l in range(N_LAYERS):
        out[f"Wq{l}"] = (_perm_w(_block_diag(wq[l])) * ISQ).astype(np.float32)
        out[f"bq{l}"] = (bq[l][PERM] * ISQ).astype(np.float32)
        out[f"Wk{l}"] = _perm_w(_block_diag(wk[l])).astype(np.float32)
        out[f"bk{l}"] = bk[l][PERM].astype(np.float32)
        out[f"Wv{l}"] = _perm_w(_block_diag(wv[l])).astype(np.float32)
        out[f"bv{l}"] = bv[l][PERM].astype(np.float32)
    out["W2"] = lin2_w[PERM].astype(np.float32)             # [64, 32]
    out["b2"] = lin2_b.astype(np.float32)
    return out


def _sel_matrices(L):
    """Sel_a [128, 16] (slices l<2), Sel_b [64, 8] (slice l=2, L==3)."""
    sel_a = np.zeros((128, 16), np.float32)
    for p in range(128):
        lp, h = p // 64, p % 8
        sel_a[p, lp * 8 + h] = 1.0
    if L == 2:
        return sel_a, None
    sel_b = np.zeros((64, 8), np.float32)
    for p in range(64):
        sel_b[p, p % 8] = 1.0
    return sel_a, sel_b


# ------------------------------------------------------- numpy device model
def simulate(inputs, fp=np.float32):
    """Numpy mirror of the device algorithm (layouts included).

    fp=np.float32 for exact-layout check; pass np_bf16-aware variant later.
    """
    ep = prep_edges(np.asarray(inputs["edge_index"]))
    W = prep_weights(*[np.asarray(inputs[k]) for k in
                       ("lin1_w", "lin1_b", "wq", "bq", "wk", "bk",
                        "wv", "bv", "lin2_w", "lin2_b")])
    rows, cols, sets = ep["rows"], ep["cols"], ep["sets"]
    EPAD, NG = ep["EPAD"], ep["NG"]

    def cast(x):
        return x.astype(fp).astype(np.float32)

    x = np.zeros((NPAD, F_IN), np.float32)
    x[:N] = np.asarray(inputs["x"], np.float32)

    # prelude: h0 (sharded then allgather — numerically identical done whole)
    h = [np.maximum(cast(cast(x) @ cast(W["W1"])) + W["b1"], 0.0)]
    h[0] = cast(h[0])

    # deg via one-hot matmuls, all cores (vectorized equivalent)
    deg = np.zeros(NPAD, np.float32)
    valid_all = cols >= 0
    np.add.at(deg, cols[valid_all], 1.0)
    degp = deg + (deg == 0)
    dis = np.sqrt(1.0 / degp).astype(np.float32)
    dis2 = (dis * dis).astype(np.float32)

    hd = [cast(np.maximum(dis2[:, None] * h[0] / np.where(dis[:, None] > 0, dis[:, None], 1), 0))]
    # hd = dis * h  (h >= 0 so relu(dis2*y) == dis*h only post-AR; here direct)
    hd[0] = cast(dis[:, None] * h[0])

    out_final = None
    for l in range(N_LAYERS):
        L = l + 1
        # ---- tables (bf16 in device; fp here) ----
        Vt = np.zeros((NPAD, 8 * max(L, 1) * 8), np.float32)  # cols d*8L+j*8+h
        for j in range(L):
            vj = cast(hd[j]) @ cast(W[f"Wv{l}"]) + dis[:, None] * W[f"bv{l}"]
            vj = cast(vj)  # cols stored (d,h)
            for d in range(8):
                Vt[:, d * 8 * L + j * 8: d * 8 * L + j * 8 + 8] = \
                    vj[:, d * 8: d * 8 + 8]
        if L > 1:
            Qt = cast(cast(h[l]) @ cast(W[f"Wq{l}"]) + W[f"bq{l}"])
            Kt = np.zeros((NPAD, 64 * L), np.float32)
            for j in range(L):
                Kt[:, 64 * j: 64 * j + 64] = cast(
                    cast(h[j]) @ cast(W[f"Wk{l}"]) + W[f"bk{l}"])

        y = np.zeros((NPAD, HID), np.float32)
        for c in range(NCORES):
            r = rows[c]
            vE = Vt[r]                          # [EPAD, 8L*8] (pad r=0 fine)
            if L > 1:
                q = Qt[np.where(cols[c] >= 0, cols[c], 0)]   # [EPAD, 64]
                k = Kt[r]                                    # [EPAD, 64L]
                # scores: s[e, lp*8+h] = sum_d q[d*8+h]*k[lp*64+d*8+h]
                s = np.zeros((EPAD, 8 * L), np.float32)
                for lp in range(L):
                    for hh in range(8):
                        s[:, lp * 8 + hh] = (
                            cast(q)[:, hh::8] * cast(k)[:, 64 * lp + hh::8][:, :8]
                        ).sum(1)
                es = np.exp(s)
                den = es.reshape(EPAD, L, 8).sum(1)          # [EPAD, 8]
                aw = cast(es * np.tile(1.0 / den, (1, L)))   # [EPAD, 8L]
                # w[e, d*8L + lp*8+h] = vE * aw
                w = cast(vE * np.tile(aw, (1, 8)))
            else:
                w = vE
            # vectorized equivalent of the per-set one-hot matmuls:
            # y[col] += sum_lp w[:, (d, lp, h)] in stored (d, h) order
            contrib = w.reshape(EPAD, 8, L, 8).sum(2).reshape(EPAD, 64)
            v = cols[c] >= 0
            np.add.at(y, cols[c][v], contrib[v])
        # epilogue: h_{l+1} = relu(dis * y), hd = relu(dis2 * y)
        hn = cast(np.maximum(dis[:, None] * y, 0.0))
        h.append(hn)
        hd.append(cast(np.maximum(dis2[:, None] * y, 0.0)))

    logits = cast(h[3]) @ cast(W["W2"]) + W["b2"]
    m = logits.max(1, keepdims=True)
    ls = logits - m - np.log(np.exp(logits - m).sum(1, keepdims=True))
    return ls[:N].astype(np.float32)



# ================================================================ bass build
def _group_sets(sets, NG):
    per_g = [[] for _ in range(NG)]
    for si, (g, b, fs, ls) in enumerate(sets):
        per_g[g].append((b, fs, ls, si))
    return per_g


def build_nc(ep, hw=True):
    import contextlib
    import concourse.bass as bass
    import concourse.mybir as mybir
    import concourse.tile as tile

    dt = mybir.dt
    AF = mybir.ActivationFunctionType
    OP = mybir.AluOpType

    EPAD, NG, sets = ep["EPAD"], ep["NG"], ep["sets"]
    NSETS = len(sets)
    per_g = _group_sets(sets, NG)
    B = BATCH                      # edges per gather batch

    nc = bass.Bass(num_devices=NCORES)
    f32, bf16, i16 = dt.float32, dt.bfloat16, dt.int16

    # ---------------- I/O ----------------
    x_sh = nc.dram_tensor("x_sh", [SHARD, F_IN], f32, kind="ExternalInput")
    rows16 = nc.dram_tensor("rows16", [128, EPAD // 16], i16, kind="ExternalInput")
    cols16 = nc.dram_tensor("cols16", [128, EPAD // 16], i16, kind="ExternalInput")
    colp_in = nc.dram_tensor("colp", [128, NSETS], f32, kind="ExternalInput")
    iota_in = nc.dram_tensor("iota", [128, 128], bf16, kind="ExternalInput")
    identb_in = nc.dram_tensor("identb", [128, 128], bf16, kind="ExternalInput")
    identf_in = nc.dram_tensor("identf", [128, 128], f32, kind="ExternalInput")
    sel2_in = nc.dram_tensor("sel2", [128, 16], bf16, kind="ExternalInput")
    sel3a_in = nc.dram_tensor("sel3a", [128, 16], bf16, kind="ExternalInput")
    sel3b_in = nc.dram_tensor("sel3b", [64, 8], bf16, kind="ExternalInput")
    W1_in = nc.dram_tensor("W1b", [F_IN, HID], bf16, kind="ExternalInput")
    b1r_in = nc.dram_tensor("b1r", [1, HID], f32, kind="ExternalInput")
    onesr_in = nc.dram_tensor("onesr", [1, 128], f32, kind="ExternalInput")
    WV0_in = nc.dram_tensor("WV0", [65, 64], bf16, kind="ExternalInput")
    WKV_in = {l: nc.dram_tensor(f"WKV{l}", [65, 128], bf16, kind="ExternalInput")
              for l in (1, 2)}
    WKVQ_in = {l: nc.dram_tensor(f"WKVQ{l}", [65, 192], bf16, kind="ExternalInput")
               for l in (1, 2)}
    W2b_in = nc.dram_tensor("W2b", [65, N_CLASS], bf16, kind="ExternalInput")
    lg_out = nc.dram_tensor("logits", [SHARD, N_CLASS], f32, kind="ExternalOutput")

    # ---------------- DRAM internals ----------------
    VW = {0: 128, 1: 128, 2: 256}
    QW = {1: 128, 2: 256}
    Vt = {l: nc.dram_tensor(f"Vt{l}", [NPAD, VW[l]], bf16) for l in range(3)}
    Kt = {l: nc.dram_tensor(f"Kt{l}", [NPAD, QW[l]], bf16) for l in (1, 2)}
    Qt = {l: nc.dram_tensor(f"Qt{l}", [NPAD, QW[l]], bf16) for l in (1, 2)}
    ybuf = nc.dram_tensor("ybuf", [NPAD, HID], f32)
    yrs = nc.dram_tensor("yrs", [SHARD, HID], f32)
    degd = nc.dram_tensor("degd", [NB, 128], f32)
    degar = nc.dram_tensor("degar", [NB, 128], f32, addr_space="Shared")
    degrs = nc.dram_tensor("degrs", [SBLK, 128], f32)
    hTs = {j: nc.dram_tensor(f"hTs{j}", [65, SHARD], bf16) for j in range(3)}
    hTf = {j: nc.dram_tensor(f"hTf{j}", [NCORES, 65, SHARD], bf16,
                             addr_space="Shared") for j in range(3)}
    RG = [list(range(NCORES))]

    with tile.TileContext(nc) as tc, contextlib.ExitStack() as ctx:
        if hw:
            # this walrus build can't encode the reload pseudo itself;
            # emit the 64-byte PSEUDO_INST(LIBRARY_RELOAD_INDEX) directly
            po = nc.isa.get_enum("NEURON_ISA_TPB_PSEUDO_OPCODE")
            nc.gpsimd.isa(
                nc.isa.Opcode.NEURON_ISA_TPB_OPCODE_PSEUDO_INST,
                {"pseudo_opcode":
                 po.NEURON_ISA_TPB_PSEUDO_OPCODE_PSEUDO_LIBRARY_RELOAD_INDEX
                 .value,
                 "lib_index": 3},
                struct_name="NEURON_ISA_TPB_PSEUDO_LIBRARY_RELOAD_INDEX_STRUCT",
                verify=False)
        else:
            from concourse import library_config
            nc.gpsimd.load_library(library_config.mlp)
        _gregs = {}

        def greg(v):
            if v not in _gregs:
                _gregs[v] = nc.gpsimd.to_reg(v)
            return _gregs[v]
        cpool = ctx.enter_context(tc.tile_pool(name="const", bufs=1))
        iota = cpool.tile([128, 128], bf16, tag="iota")
        identb = cpool.tile([128, 128], bf16, tag="identb")
        identf = cpool.tile([128, 128], f32, tag="identf")
        sel2 = cpool.tile([128, 16], bf16, tag="sel2")
        sel3a = cpool.tile([128, 16], bf16, tag="sel3a")
        sel3b = cpool.tile([64, 8], bf16, tag="sel3b")
        W1t = cpool.tile([128, 2, HID], bf16, tag="W1t")
        b1tile = cpool.tile([128, HID], f32, tag="b1tile")
        WV0 = cpool.tile([65, 64], bf16, tag="WV0")
        WKV = {l: cpool.tile([65, 128], bf16, tag=f"WKV{l}", name=f"WKV{l}") for l in (1, 2)}
        WKVQ = {l: cpool.tile([65, 192], bf16, tag=f"WKVQ{l}", name=f"WKVQ{l}") for l in (1, 2)}
        W2b = cpool.tile([65, N_CLASS], bf16, tag="W2b")
        rows_sb = cpool.tile([128, EPAD // 16], i16, tag="rows_sb")
        cols_sb = cpool.tile([128, EPAD // 16], i16, tag="cols_sb")
        colp_sb = cpool.tile([128, NSETS], f32, tag="colp_sb")
        disfull = cpool.tile([128, NB], f32, tag="disfull")
        dissh = cpool.tile([128, SBLK], f32, tag="dissh")
        dis2sh = cpool.tile([128, SBLK], f32, tag="dis2sh")
        h0rows = cpool.tile([128, SBLK, 65], bf16, tag="h0rows")

        nc.sync.dma_start(out=iota[:], in_=iota_in[:])
        nc.sync.dma_start(out=identb[:], in_=identb_in[:])
        nc.sync.dma_start(out=identf[:], in_=identf_in[:])
        nc.sync.dma_start(out=sel2[:], in_=sel2_in[:])
        nc.sync.dma_start(out=sel3a[:], in_=sel3a_in[:])
        nc.sync.dma_start(out=sel3b[:], in_=sel3b_in[:])
        nc.sync.dma_start(out=W1t[:, 0, :], in_=W1_in[0:128, :])
        nc.sync.dma_start(out=W1t[:, 1, :], in_=W1_in[128:256, :])
        nc.sync.dma_start(out=WV0[:], in_=WV0_in[:])
        for l in (1, 2):
            nc.sync.dma_start(out=WKV[l][:], in_=WKV_in[l][:])
            nc.sync.dma_start(out=WKVQ[l][:], in_=WKVQ_in[l][:])
        nc.sync.dma_start(out=W2b[:], in_=W2b_in[:])
        nc.sync.dma_start(out=rows_sb[:], in_=rows16[:])
        nc.sync.dma_start(out=cols_sb[:], in_=cols16[:])
        nc.sync.dma_start(out=colp_sb[:], in_=colp_in[:])

        scope_prelude = contextlib.ExitStack()
        scope_prelude.enter_context(nc.named_scope("prelude"))
        preludeA = contextlib.ExitStack()
        psA = preludeA.enter_context(tc.tile_pool(name="psA", bufs=2, space="PSUM"))
        sbA = preludeA.enter_context(tc.tile_pool(name="sbA", bufs=3))

        # b1 broadcast tile via K=1 matmul (ones_col x b1_row)
        onesr_sb = cpool.tile([1, 128], f32, tag="onesr")
        b1r_sb = cpool.tile([1, HID], f32, tag="b1r")
        nc.sync.dma_start(out=onesr_sb[:], in_=onesr_in[:])
        nc.sync.dma_start(out=b1r_sb[:], in_=b1r_in[:])
        b1p = psA.tile([128, HID], f32, tag="b1p")
        nc.tensor.matmul(out=b1p[:], lhsT=onesr_sb[:], rhs=b1r_sb[:],
                         start=True, stop=True)
        nc.vector.tensor_copy(out=b1tile[:], in_=b1p[:])

        # ---------------- P1: h0 shard + transpose + allgather
        for t in range(SBLK):
            xf = sbA.tile([128, F_IN], f32, tag="xf")
            nc.sync.dma_start(out=xf[:], in_=x_sh[128 * t:128 * t + 128, :])
            xb = sbA.tile([128, F_IN], bf16, tag="xb")
            nc.vector.tensor_copy(out=xb[:], in_=xf[:])
            xT = sbA.tile([128, 2, 128], bf16, tag="xT")
            for k in range(2):
                tp = psA.tile([128, 128], bf16, tag="tp")
                nc.tensor.transpose(out=tp[:], in_=xb[:, 128 * k:128 * k + 128],
                                    identity=identb[:])
                nc.vector.tensor_copy(out=xT[:, k, :], in_=tp[:])
            hp = psA.tile([128, HID], f32, tag="hp")
            nc.tensor.matmul(out=hp[:], lhsT=xT[:, 0, :], rhs=W1t[:, 0, :],
                             start=True, stop=False)
            nc.tensor.matmul(out=hp[:], lhsT=xT[:, 1, :], rhs=W1t[:, 1, :],
                             start=False, stop=True)
            hs = sbA.tile([128, HID], f32, tag="hs")
            nc.vector.tensor_add(out=hs[:], in0=hp[:], in1=b1tile[:])
            nc.scalar.activation(out=h0rows[:, t, 0:64], in_=hs[:], func=AF.Relu)
            nc.vector.memset(h0rows[:, t, 64:65], 1.0)
        for t in range(SBLK):
            tp = psA.tile([65, 128], bf16, tag="tp65")
            nc.tensor.transpose(out=tp[:], in_=h0rows[:, t, :],
                                identity=identb[:])
            hT = sbA.tile([65, 128], bf16, tag="hTsb")
            nc.vector.tensor_copy(out=hT[:], in_=tp[:])
            nc.sync.dma_start(out=hTs[0][:, 128 * t:128 * t + 128], in_=hT[:])
        nc.gpsimd.collective_compute(
            "AllGather", OP.bypass, replica_groups=RG,
            ins=[hTs[0][:].opt()], outs=[hTf[0][:].opt()])

        preludeA.close()

        # ---------------- P1b: degree one-hot matmuls (row staging)
        onescol = cpool.tile([128, 1], bf16, tag="onescol")
        nc.vector.memset(onescol[:], 1.0)
        DGB = 8  # blocks per staging row (NB % DGB == 0)
        with tc.tile_pool(name="ohp", bufs=6) as ohp, \
             tc.tile_pool(name="dgrow", bufs=2) as dgrow, \
             tc.tile_pool(name="dgp", bufs=4, space="PSUM") as dgp:
            dcur = {}
            drow = None
            for si, (g, b, fs, ls) in enumerate(sets):
                oh = ohp.tile([128, 128], bf16, tag="oh")
                nc.vector.tensor_scalar(
                    out=oh[:], in0=iota[:], scalar1=colp_sb[:, si:si + 1],
                    scalar2=None, op0=OP.is_equal)
                if fs:
                    dcur[b] = dgp.tile([1, 128], f32, tag="dg", name="dg")
                nc.tensor.matmul(out=dcur[b][:], lhsT=onescol[:], rhs=oh[:],
                                 start=fs, stop=ls)
                if ls:
                    if b % DGB == 0:
                        drow = dgrow.tile([1, DGB * 128], f32, tag="drow",
                                          name="drow")
                    nc.vector.tensor_copy(
                        out=drow[0:1, 128 * (b % DGB):128 * (b % DGB) + 128],
                        in_=dcur[b][:])
                    del dcur[b]
                    if b % DGB == DGB - 1:
                        nc.sync.dma_start(
                            out=degd[b - DGB + 1:b + 1, :], in_=drow[:])
        nc.gpsimd.collective_compute(
            "AllReduce", OP.add, replica_groups=RG,
            ins=[degd[:].opt()], outs=[degar[:].opt()])
        nc.gpsimd.collective_compute(
            "ReduceScatter", OP.add, replica_groups=RG,
            ins=[degd[:].opt()], outs=[degrs[:].opt()])

        # disfull [128 node, NB] from degar; dissh/dis2sh from degrs
        with tc.tile_pool(name="dsb", bufs=2) as dsb, \
             tc.tile_pool(name="dps", bufs=2, space="PSUM") as dps:
            for half in range((NB + 127) // 128):
                nbh = min(128, NB - 128 * half)
                dgt = dsb.tile([128, 128], f32, tag="dgt")
                nc.sync.dma_start(out=dgt[:nbh, :],
                                  in_=degar[128 * half:128 * half + nbh, :])
                # fix deg==0 -> 1, then reciprocal & sqrt
                z = dsb.tile([128, 128], f32, tag="z")
                nc.vector.tensor_scalar(out=z[:nbh, :], in0=dgt[:nbh, :],
                                        scalar1=0.0, scalar2=None,
                                        op0=OP.is_equal)
                nc.vector.tensor_add(out=dgt[:nbh, :], in0=dgt[:nbh, :],
                                     in1=z[:nbh, :])
                nc.vector.reciprocal(out=dgt[:nbh, :], in_=dgt[:nbh, :])
                nc.scalar.activation(out=dgt[:nbh, :], in_=dgt[:nbh, :],
                                     func=AF.Sqrt)
                tp = dps.tile([128, 128], f32, tag="dtp")
                nc.tensor.transpose(out=tp[:, :nbh], in_=dgt[:nbh, :],
                                    identity=identf[:nbh, :nbh])
                nc.vector.tensor_copy(
                    out=disfull[:, 128 * half:128 * half + nbh],
                    in_=tp[:, :nbh])
            # shard dis / dis^2
            dgs = dsb.tile([SBLK, 128], f32, tag="dgs")
            nc.sync.dma_start(out=dgs[:], in_=degrs[:])
            z2 = dsb.tile([SBLK, 128], f32, tag="z2")
            nc.vector.tensor_scalar(out=z2[:], in0=dgs[:], scalar1=0.0,
                                    scalar2=None, op0=OP.is_equal)
            nc.vector.tensor_add(out=dgs[:], in0=dgs[:], in1=z2[:])
            nc.vector.reciprocal(out=dgs[:], in_=dgs[:])  # = dis^2
            tp = dps.tile([128, SBLK], f32, tag="dtp2")
            nc.tensor.transpose(out=tp[:], in_=dgs[:],
                                identity=identf[:SBLK, :SBLK])
            nc.vector.tensor_copy(out=dis2sh[:], in_=tp[:])
            nc.scalar.activation(out=dgs[:], in_=dgs[:], func=AF.Sqrt)
            tp2 = dps.tile([128, SBLK], f32, tag="dtp2")
            nc.tensor.transpose(out=tp2[:], in_=dgs[:],
                                identity=identf[:SBLK, :SBLK])
            nc.vector.tensor_copy(out=dissh[:], in_=tp2[:])

        scope_prelude.close()

        # ================= per-layer helpers (closure over tc/nc) =========
        def build_tables(l):
            L = l + 1
            with tc.tile_pool(name=f"slab{l}", bufs=2) as slp, \
                 tc.tile_pool(name=f"tps{l}", bufs=2, space="PSUM") as tps, \
                 tc.tile_pool(name=f"tstg{l}", bufs=3) as stg:
                for s in range(NCORES):
                    slabs = []
                    for j in range(L):
                        sl = slp.tile([65, SHARD], bf16, tag=f"slab{j}")
                        nc.sync.dma_start(out=sl[:], in_=hTf[j][s, :, :])
                        slabs.append(sl)
                    for lb in range(SBLK):
                        bb = SBLK * s + lb
                        ck = slice(128 * lb, 128 * lb + 128)
                        dcol = disfull[:, bb:bb + 1]
                        if l == 0:
                            vp = tps.tile([128, 64], f32, tag="vp")
                            nc.tensor.matmul(out=vp[:], lhsT=slabs[0][:, ck],
                                             rhs=WV0[:], start=True, stop=True)
                            vstg = stg.tile([128, VW[0]], bf16, tag="vstg")
                            nc.vector.memset(vstg[:, 64:128], 0)
                            nc.vector.tensor_tensor(
                                out=vstg[:, 0:64], in0=vp[:],
                                in1=dcol.to_broadcast([128, 64]),
                                op=OP.mult)
                            nc.sync.dma_start(
                                out=Vt[0][128 * bb:128 * bb + 128, :],
                                in_=vstg[:])
                        else:
                            kstg = stg.tile([128, QW[l]], bf16, tag="kstg")
                            vstg = stg.tile([128, VW[l]], bf16, tag="vstg")
                            qstg = stg.tile([128, QW[l]], bf16, tag="qstg")
                            if l == 2:
                                nc.vector.memset(kstg[:, 192:256], 0)
                                nc.vector.memset(vstg[:, 192:256], 0)
                            vstg3 = vstg[:, 0:64 * L].rearrange(
                                "p (d l h) -> p d l h", d=8, l=L, h=8)
                            for j in range(L):
                                if j < l:
                                    pj = tps.tile([128, 128], f32, tag="pkv")
                                    nc.tensor.matmul(
                                        out=pj[:], lhsT=slabs[j][:, ck],
                                        rhs=WKV[l][:], start=True, stop=True)
                                else:
                                    pj = tps.tile([128, 192], f32, tag="pkvq")
                                    nc.tensor.matmul(
                                        out=pj[:], lhsT=slabs[j][:, ck],
                                        rhs=WKVQ[l][:], start=True, stop=True)
                                nc.vector.tensor_copy(
                                    out=kstg[:, 64 * j:64 * j + 64],
                                    in_=pj[:, 0:64])
                                nc.vector.tensor_tensor(
                                    out=vstg3[:, :, j, :],
                                    in0=pj[:, 64:128].rearrange(
                                        "p (d h) -> p d h", d=8, h=8),
                                    in1=dcol.to_broadcast([128, 8, 8]),
                                    op=OP.mult)
                                if j == l:
                                    nreps = QW[l] // 64
                                    nc.vector.tensor_copy(
                                        out=qstg[:].rearrange(
                                            "p (r f) -> p r f", r=nreps),
                                        in_=pj[:, 128:192][:, None, :]
                                        .to_broadcast([128, nreps, 64]))
                            nc.sync.dma_start(
                                out=Kt[l][128 * bb:128 * bb + 128, :],
                                in_=kstg[:])
                            nc.sync.dma_start(
                                out=Vt[l][128 * bb:128 * bb + 128, :],
                                in_=vstg[:])
                            nc.sync.dma_start(
                                out=Qt[l][128 * bb:128 * bb + 128, :],
                                in_=qstg[:])

        def edge_pass(l):
            L = l + 1
            SW = 8 * L
            nbat = (EPAD + B - 1) // B
            with tc.tile_pool(name=f"gth{l}", bufs=2) as gth, \
                 tc.tile_pool(name=f"cmp{l}", bufs=2) as cmp, \
                 tc.tile_pool(name=f"scp{l}", bufs=2, space="PSUM") as scps, \
                 tc.tile_pool(name=f"agg{l}", bufs=4, space="PSUM") as aggs, \
                 tc.tile_pool(name=f"ohe{l}", bufs=6) as ohe, \
                 tc.tile_pool(name=f"yp{l}", bufs=3) as yp:
                aggcur = {}
                for bi in range(nbat):
                    e0 = bi * B
                    bs = min(B, EPAD - e0)
                    gb = bs // 128
                    idr = rows_sb[:, e0 // 16:(e0 + bs) // 16]
                    vE = gth.tile([128, bs // 128, VW[l]], bf16, tag="vE")
                    nc.gpsimd.dma_gather(
                        out_ap=vE[:], in_ap=Vt[l][:], idxs_ap=idr,
                        num_idxs=bs, num_idxs_reg=greg(bs), elem_size=VW[l],
                        single_packet=False)
                    if L > 1:
                        jj = QW[l] // 128
                        kT = gth.tile([128, jj, bs], bf16, tag="kT")
                        qT = gth.tile([128, jj, bs], bf16, tag="qT")
                        nc.gpsimd.dma_gather(
                            out_ap=kT[:], in_ap=Kt[l][:], idxs_ap=idr,
                            num_idxs=bs, num_idxs_reg=greg(bs), elem_size=QW[l],
                            transpose=True, single_packet=False)
                        nc.gpsimd.dma_gather(
                            out_ap=qT[:], in_ap=Qt[l][:],
                            idxs_ap=cols_sb[:, e0 // 16:(e0 + bs) // 16],
                            num_idxs=bs, num_idxs_reg=greg(bs), elem_size=QW[l],
                            transpose=True, single_packet=False)
                        prod = cmp.tile([128, jj, bs], bf16, tag="prod")
                        nc.vector.tensor_tensor(out=prod[:], in0=kT[:],
                                                in1=qT[:], op=OP.mult)
                        scp = scps.tile([128, gb * 32], f32, tag="scp")
                        for c in range(gb):
                            cs = slice(128 * c, 128 * c + 128)
                            nc.tensor.matmul(
                                out=scp[:, 32 * c:32 * c + 16],
                                lhsT=prod[:, 0, cs],
                                rhs=sel2[:] if L == 2 else sel3a[:],
                                start=True, stop=True)
                            if L == 3:
                                nc.tensor.matmul(
                                    out=scp[:, 32 * c + 16:32 * c + 24],
                                    lhsT=prod[0:64, 1, cs], rhs=sel3b[:],
                                    start=True, stop=True)
                        esc = cmp.tile([128, gb, SW], f32, tag="esc")
                        nc.scalar.activation(
                            out=esc[:],
                            in_=scp[:].rearrange("p (g t) -> p g t", t=32)
                            [:, :, 0:SW],
                            func=AF.Exp)
                        den = cmp.tile([128, gb, 8], f32, tag="den")
                        nc.vector.tensor_reduce(
                            out=den[:],
                            in_=esc[:].rearrange("p g (l h) -> p g h l", l=L),
                            axis=mybir.AxisListType.X, op=OP.add)
                        rec = cmp.tile([128, gb, 8], f32, tag="rec")
                        nc.vector.reciprocal(out=rec[:], in_=den[:])
                        aw = cmp.tile([128, gb, L, 8], bf16, tag="aw")
                        for lp in range(L):
                            nc.vector.tensor_tensor(
                                out=aw[:, :, lp, :],
                                in0=esc[:].rearrange(
                                    "p g (l h) -> p g l h", l=L)[:, :, lp, :],
                                in1=rec[:], op=OP.mult)
                        # w[p, g, d, (l h)] = vE * aw  (3 free dims max)
                        w = cmp.tile([128, gb, 8, L * 8], bf16, tag="w")
                        nc.vector.tensor_tensor(
                            out=w[:],
                            in0=vE[:, :, 0:64 * L].rearrange(
                                "p g (d lh) -> p g d lh", d=8),
                            in1=aw[:].rearrange("p g l h -> p g (l h)")
                            [:, :, None, :].to_broadcast(
                                [128, gb, 8, L * 8]),
                            op=OP.mult)
                    for c in range(gb):
                        g = e0 // 128 + c
                        for (b, fs, ls, si) in per_g[g]:
                            oh = ohe.tile([128, 128], bf16, tag="oh")
                            nc.vector.tensor_scalar(
                                out=oh[:], in0=iota[:],
                                scalar1=colp_sb[:, si:si + 1], scalar2=None,
                                op0=OP.is_equal)
                            if fs:
                                aggcur[b] = aggs.tile([128, HID], f32, tag="aggp", name="aggp")
                            for lp in range(L):
                                rhs = (vE[:, c, 0:64] if L == 1 else
                                       w[:, c, :, 8 * lp:8 * lp + 8])
                                nc.tensor.matmul(
                                    out=aggcur[b][:], lhsT=oh[:], rhs=rhs,
                                    start=(fs and lp == 0),
                                    stop=(ls and lp == L - 1))
                            if ls:
                                ysb = yp.tile([128, HID], f32, tag="ysb")
                                nc.scalar.activation(out=ysb[:],
                                                     in_=aggcur[b][:],
                                                     func=AF.Copy)
                                nc.sync.dma_start(
                                    out=ybuf[128 * b:128 * b + 128, :],
                                    in_=ysb[:])
                                del aggcur[b]

        def post_ar(l):
            nc.gpsimd.collective_compute(
                "ReduceScatter", OP.add, replica_groups=RG,
                ins=[ybuf[:].opt()], outs=[yrs[:].opt()])
            with tc.tile_pool(name=f"par{l}", bufs=3) as par, \
                 tc.tile_pool(name=f"pps{l}", bufs=2, space="PSUM") as pps:
                for lb in range(SBLK):
                    ysh = par.tile([128, HID], f32, tag="ysh")
                    nc.sync.dma_start(
                        out=ysh[:], in_=yrs[128 * lb:128 * lb + 128, :])
                    hrow = par.tile([128, 65], bf16, tag="hrow")
                    nc.scalar.activation(out=hrow[:, 0:64], in_=ysh[:],
                                         func=AF.Relu,
                                         scale=dissh[:, lb:lb + 1])
                    nc.vector.memset(hrow[:, 64:65], 1.0)
                    tp = pps.tile([65, 128], bf16, tag="tp65b")
                    nc.tensor.transpose(out=tp[:], in_=hrow[:],
                                        identity=identb[:])
                    hT = par.tile([65, 128], bf16, tag="hTsb2")
                    nc.vector.tensor_copy(out=hT[:], in_=tp[:])
                    if l < 2:
                        nc.sync.dma_start(
                            out=hTs[l + 1][:, 128 * lb:128 * lb + 128],
                            in_=hT[:])
                    else:
                        lgp = pps.tile([128, N_CLASS], f32, tag="lgp")
                        nc.tensor.matmul(out=lgp[:], lhsT=hT[:], rhs=W2b[:],
                                         start=True, stop=True)
                        mx = par.tile([128, 1], f32, tag="mx")
                        nc.vector.tensor_reduce(out=mx[:], in_=lgp[:],
                                                axis=mybir.AxisListType.X,
                                                op=OP.max)
                        t1 = par.tile([128, N_CLASS], f32, tag="t1")
                        nc.vector.tensor_scalar(
                            out=t1[:], in0=lgp[:], scalar1=mx[:],
                            scalar2=None, op0=OP.subtract)
                        ex = par.tile([128, N_CLASS], f32, tag="ex")
                        sm = par.tile([128, 1], f32, tag="sm")
                        nc.scalar.activation(out=ex[:], in_=t1[:],
                                             func=AF.Exp, accum_out=sm[:])
                        lns = par.tile([128, 1], f32, tag="lns")
                        nc.scalar.activation(out=lns[:], in_=sm[:],
                                             func=AF.Ln)
                        lgo = par.tile([128, N_CLASS], f32, tag="lgo")
                        nc.vector.tensor_scalar(
                            out=lgo[:], in0=t1[:], scalar1=lns[:],
                            scalar2=None, op0=OP.subtract)
                        nc.sync.dma_start(
                            out=lg_out[128 * lb:128 * lb + 128, :],
                            in_=lgo[:])
                if l < 2:
                    nc.gpsimd.collective_compute(
                        "AllGather", OP.bypass, replica_groups=RG,
                        ins=[hTs[l + 1][:].opt()],
                        outs=[hTf[l + 1][:].opt()])

        # ---------------- layers ----------------
        for l in range(N_LAYERS):
            with nc.named_scope(f"tables{l}"):
                build_tables(l)
            with nc.named_scope(f"edges{l}"):
                edge_pass(l)
            with nc.named_scope(f"post{l}"):
                post_ar(l)

    # this walrus build allows at most ONE sync-wait command per
    # instruction; split excess waits onto inserted drains
    nsplit = 0
    for bb in nc.main_func.blocks:
        out = []
        for ins in list(bb.instructions):
            si = ins.sync_info
            if si is not None and si.on_wait and len(si.on_wait) > 1:
                waits = list(si.on_wait)
                k = 0
                while len(waits) > 1:
                    chunk, waits = waits[:1], waits[1:]
                    nop = mybir.InstDrain(
                        name=f"{ins.name}_ws{k}", engine=ins.engine,
                        ins=[], outs=[],
                        sync_info=mybir.SyncInfo(on_wait=chunk, on_update=[]))
                    nc.register_instruction(nop)
                    out.append(nop)
                    k += 1
                    nsplit += 1
                si.on_wait = waits
            out.append(ins)
        bb.instructions = out
    return nc


# ================================================================ entry
def _build_inmaps(inputs, ep):
    W = prep_weights(*[np.asarray(inputs[k]) for k in
                       ("lin1_w", "lin1_b", "wq", "bq", "wk", "bk",
                        "wv", "bv", "lin2_w", "lin2_b")])
    rows, cols, sets = ep["rows"], ep["cols"], ep["sets"]
    NSETS = len(sets)

    xpad = np.zeros((NPAD, F_IN), np.float32)
    xpad[:N] = np.asarray(inputs["x"], np.float32)

    sel_a3, sel_b3 = _sel_matrices(3)
    sel_a2, _ = _sel_matrices(2)

    def stackb(w, b):
        return np.concatenate([w, b[None, :]], 0).astype(np_bf16)

    common = {
        "iota": np.tile(np.arange(128, dtype=np.float32)[None, :],
                        (128, 1)).astype(np_bf16),
        "identb": np.eye(128, dtype=np.float32).astype(np_bf16),
        "identf": np.eye(128, dtype=np.float32),
        "sel2": sel_a2.astype(np_bf16),
        "sel3a": sel_a3.astype(np_bf16),
        "sel3b": sel_b3.astype(np_bf16),
        "W1b": W["W1"].astype(np_bf16),
        "b1r": W["b1"][None, :].astype(np.float32),
        "onesr": np.ones((1, 128), np.float32),
        "WV0": stackb(W["Wv0"], W["bv0"]),
        "W2b": stackb(W["W2"], W["b2"]),
    }
    for l in (1, 2):
        kv = np.concatenate(
            [np.concatenate([W[f"Wk{l}"], W[f"bk{l}"][None]], 0),
             np.concatenate([W[f"Wv{l}"], W[f"bv{l}"][None]], 0)], 1)
        common[f"WKV{l}"] = kv.astype(np_bf16)
        common[f"WKVQ{l}"] = np.concatenate(
            [kv, np.concatenate([W[f"Wq{l}"], W[f"bq{l}"][None]], 0)],
            1).astype(np_bf16)

    in_maps = []
    for c in range(NCORES):
        colp = np.zeros((128, NSETS), np.float32)
        for si, (g, b, fs, ls) in enumerate(sets):
            colp[:, si] = cols[c, 128 * g:128 * g + 128] - 128 * b
        m = dict(common)
        m["x_sh"] = xpad[SHARD * c:SHARD * (c + 1)].copy()
        m["rows16"] = _wrap16(np.maximum(rows[c], 0))
        m["cols16"] = _wrap16(np.maximum(cols[c], 0))
        m["colp"] = colp.astype(np.float32)
        in_maps.append(m)
    return in_maps


_CACHE = {}


def _patch_interp():
    """Teach bass_interp's InstISA visitor the PSEUDO_INST library reload
    (opcode 223) that we emit as raw bytes for walrus compatibility."""
    from concourse import bass_interp
    if getattr(bass_interp, "_dna_isa_patch", False):
        return
    orig = bass_interp._visit_InstISA

    def patched(isa, instruction, core_sim):
        if instruction.isa_opcode == 223:
            lib = (instruction.ant_dict or {}).get("lib_index", 0)
            core_sim.pool_library_index = lib
            return
        return orig(isa, instruction, core_sim)

    bass_interp._visit_InstISA = patched
    bass_interp._dna_isa_patch = True


def kernel(**inputs):
    import sys
    if "/opt/trn_rl_repo" not in sys.path:
        sys.path.insert(0, "/opt/trn_rl_repo")
    from concourse.bass_utils import run_bass_kernel_spmd
    _patch_interp()

    import os
    ep = prep_edges(np.asarray(inputs["edge_index"]))
    in_maps = _build_inmaps(inputs, ep)
    nc = build_nc(ep)
    trace = bool(os.environ.get("DNA_TRACE"))
    res = run_bass_kernel_spmd(nc, in_maps, core_ids=list(range(NCORES)),
                               trace=trace)
    _CACHE["res"] = res
    logits = np.concatenate([res.results[c]["logits"]
                             for c in range(NCORES)], 0)
    return logits[:N].astype(np.float32)

